# revision 41
# baseline (speedup 1.0000x reference)
"""LoRA linear layer (out = x @ (W + s*A@B) + bias) on 8 Trainium2 NeuronCores.

Sharding: data-parallel over rows of x (M = 4*2048 = 8192 -> 1024 rows/core);
each core computes its row-slice against the full weight matrix.

Per-core kernel: fp8 (e4m3) matmuls in DoubleRow perf mode (2 k-groups of 128
per instruction, 2 MACs/cycle/lane) with a hi/lo split for accuracy:

  64*x@W ~= x_hi@W_hi + x_lo@W_hi + x_hi@W_lo      (W_* store 64*W in fp8)

Three half-cost matmuls replace one full-cost fp32r/bf16 matmul (0.75x PE
time), with quantization error ~1.5e-3 max-rel (gate is 2e-2).
The x_lo@W_lo term (~1e-4) is dropped.

LoRA path:
  - xat = 64*(x @ A), rank 16, via the same 3-term fp8 DoubleRow split
    (A scaled by 64 and split hi/lo); plain-cast to bf16 on the vector
    engine (the 64 cancels against B's s/1 pre-scale at the 1/64 drain)
  - per out tile, one rank-16 bf16 matmul adds it into the same PSUM
    accumulation, emitted mid-group so it stays off the drain tail

Output is computed transposed [d_out, m] in f16; the PSUM -> SBUF drain on
the scalar engine applies the 1/64 descale and the per-channel bias; the
host transposes back and upcasts. A fused first sweep computes 5 output
tiles + xat while the x hi/lo stream lands in 2-k-pair chunks (DMA issue
alternating between the SP and ACT sequencers) so the PE never starves;
later weight tiles prefetch one 256-column group ahead. Throwaway warmup
matmuls on a zeroed scratch tile burn the cold-clock ramp during the
initial DMA wait.
"""
import numpy as np
import ml_dtypes

import concourse.tile as tile
from concourse import bacc, mybir
from concourse.bass_utils import run_bass_kernel_spmd

P = 128
N_CORES = 8
BATCH, SEQ = 4, 2048
D_IN, D_OUT, RANK = 4096, 4096, 16
M_FULL = BATCH * SEQ          # 8192
M_C = M_FULL // N_CORES       # 1024 rows per core
KP = D_IN // (2 * P)          # 16 k-pairs (DoubleRow consumes 256 rows)
MC = M_C // 512               # 2 moving chunks of 512
NTP = D_OUT // 256            # 16 n-groups (W loaded 256 cols at a time)
NT = D_OUT // P               # 32 n-tiles
F32 = mybir.dt.float32
F16 = mybir.dt.float16
BF16 = mybir.dt.bfloat16
F8 = mybir.dt.float8e4
NPF8 = ml_dtypes.float8_e4m3
SW = 64.0                     # W/B scale folded out in the drain
SA = 64.0                     # lora_A scale folded out in the xat copy
DR = mybir.MatmulPerfMode.DoubleRow
# (n-tile, m-chunk) pairs fused into the x-landing sweep: 5 tiles + 2 xps
# accumulators = 7 PSUM banks, leaving one free so the first main-loop tile
# can start while the sweep's LoRA/drain chain completes
SWEEP_PAIRS = [(0, 0), (0, 1), (1, 0), (1, 1), (2, 0)]

_NC_CACHE = None


def _emit_body(nc, pools, aps, sb, rep):
    singles, w_pool, out_pool, psum_pool = pools
    xh_d, xl_d, wh_d, wl_d, ah_d, al_d, bb_d, bias_d, outt_d = aps
    xh, xl, ah_sb, al_sb, bb_sb, xat, bias_sb = (
        sb["xh"], sb["xl"], sb["ah_sb"], sb["al_sb"], sb["bb_sb"],
        sb["xat"], sb["bias_sb"])

    n_dma = [0]

    def dma(out, in_):
        eng = nc.sync if n_dma[0] % 2 == 0 else nc.scalar
        n_dma[0] += 1
        eng.dma_start(out=out, in_=in_)

    def mm3(ps, wh_t, wl_t, kp, sub, mc, start, stop=False):
        """The three hi/lo product terms for one (out tile, k-pair)."""
        nsl = slice(sub * P, (sub + 1) * P)
        msl = slice(mc * 512, (mc + 1) * 512)
        nc.tensor.matmul(ps, wh_t[:, kp, :, nsl], xh[:, kp, :, msl],
                         start=start, stop=False, perf_mode=DR)
        nc.tensor.matmul(ps, wh_t[:, kp, :, nsl], xl[:, kp, :, msl],
                         start=False, stop=False, perf_mode=DR)
        nc.tensor.matmul(ps, wl_t[:, kp, :, nsl], xh[:, kp, :, msl],
                         start=False, stop=stop, perf_mode=DR)

    def b_apply(ps, nt, mc, stop):
        nc.tensor.matmul(ps, bb_sb[:, nt * P:(nt + 1) * P],
                         xat[:, mc * 512:(mc + 1) * 512],
                         start=False, stop=stop)

    def drain(ps, nt, mc, tag):
        """descale/bias PSUM->SBUF + store."""
        ob = out_pool.tile([P, 512], F16, tag="ob", name=f"ob_{rep}_{tag}")
        nc.scalar.activation(ob, ps, mybir.ActivationFunctionType.Identity,
                             bias=bias_sb[:, nt:nt + 1], scale=1.0 / SW)
        nc.sync.dma_start(
            out=outt_d[nt * P:(nt + 1) * P, mc * 512:(mc + 1) * 512], in_=ob)

    def w_tiles(ntp):
        wh_t = w_pool.tile([P, KP, 2, 256], F8, tag="wt", name=f"wh_{rep}_{ntp}")
        dma(wh_t, wh_d[:, ntp])
        wl_t = w_pool.tile([P, KP, 2, 256], F8, tag="wt", name=f"wl_{rep}_{ntp}")
        dma(wl_t, wl_d[:, ntp])
        return wh_t, wl_t

    # ---- fused first sweep: x stream + xat + SWEEP_PAIRS out tiles ----
    # inputs stream in 2-k-pair groups in first-use order so the PE starts
    # after the first ~0.7MB instead of the full ntp0/ntp1 weight load;
    # issue alternates between the SP and ACT sequencers (HWDGE is shared
    # but the ~1.2us per-DMA sequencer cost is not)
    w0 = (w_pool.tile([P, KP, 2, 256], F8, tag="wt", name=f"wh_{rep}_0"),
          w_pool.tile([P, KP, 2, 256], F8, tag="wt", name=f"wl_{rep}_0"))
    w1 = (w_pool.tile([P, KP, 2, 256], F8, tag="wt", name=f"wh_{rep}_1"),
          w_pool.tile([P, KP, 2, 256], F8, tag="wt", name=f"wl_{rep}_1"))
    groups = [slice(0, 1), slice(1, 2)] + [
        slice(2 * g, 2 * g + 2) for g in range(1, KP // 2)]
    for gi, ks in enumerate(groups):
        dma(xh[:, ks], xh_d[:, ks])
        dma(w0[0][:, ks], wh_d[:, 0, ks])
        dma(w1[0][:, ks], wh_d[:, 1, ks])
        dma(xl[:, ks], xl_d[:, ks])
        dma(w0[1][:, ks], wl_d[:, 0, ks])
        dma(w1[1][:, ks], wl_d[:, 1, ks])
        if gi == 0:
            dma(ah_sb, ah_d)
            dma(al_sb, al_d)
        if gi == 4:
            dma(bb_sb, bb_d)
            dma(bias_sb, bias_d)
    sweep = SWEEP_PAIRS
    ps_sw = {(nt, mc): psum_pool.tile([P, 512], F32, tag="ps",
                                      name=f"ps_{rep}_{nt}_{mc}")
             for nt, mc in sweep}
    xps = [psum_pool.tile([P, 512], F32, tag="ps", name=f"xp_{rep}_{mc}")
           for mc in range(MC)]
    for kp in range(KP):
        for term in range(3):
            for nt, mc in sweep:
                wh_t, wl_t = (w0, w1)[nt // 2]
                nsl = slice((nt % 2) * P, (nt % 2 + 1) * P)
                msl = slice(mc * 512, (mc + 1) * 512)
                w_op = (wh_t[:, kp, :, nsl], wh_t[:, kp, :, nsl],
                        wl_t[:, kp, :, nsl])[term]
                x_op = (xh[:, kp, :, msl], xl[:, kp, :, msl],
                        xh[:, kp, :, msl])[term]
                nc.tensor.matmul(ps_sw[(nt, mc)], w_op, x_op,
                                 start=(kp == 0 and term == 0), stop=False,
                                 perf_mode=DR)
        for mc in range(MC):
            msl = slice(mc * 512, (mc + 1) * 512)
            nc.tensor.matmul(xps[mc][0:RANK, :], ah_sb[:, kp],
                             xh[:, kp, :, msl],
                             start=(kp == 0), stop=False, perf_mode=DR)
            nc.tensor.matmul(xps[mc][0:RANK, :], ah_sb[:, kp],
                             xl[:, kp, :, msl],
                             start=False, stop=False, perf_mode=DR)
            nc.tensor.matmul(xps[mc][0:RANK, :], al_sb[:, kp],
                             xh[:, kp, :, msl],
                             start=False, stop=(kp == KP - 1), perf_mode=DR)
    # xat keeps the SA scale (cancelled by lora_B's SW/SA pre-scale), so the
    # PSUM->SBUF copy is a plain cast on the otherwise-idle vector engine
    for mc in range(MC):
        nc.vector.tensor_copy(
            out=xat[:, mc * 512:(mc + 1) * 512], in_=xps[mc][0:RANK, :])

    # ---- main loop over remaining (n-tile, m-chunk) pairs ----
    # the first pair's matmuls are emitted before the sweep drains so the PE
    # rolls straight from the last xps matmul into main work while the
    # xat copy / B-apply / drain chain of the sweep tiles completes
    wts = {0: w0, 1: w1}
    remaining = [(nt, mc) for nt in range(NT) for mc in range(MC)
                 if (nt, mc) not in sweep]
    for i, (nt, mc) in enumerate(remaining):
        ntp = nt // 2
        if ntp + 1 < NTP and (ntp + 1) not in wts:
            wts[ntp + 1] = w_tiles(ntp + 1)
        wh_t, wl_t = wts[ntp]
        ps = psum_pool.tile([P, 512], F32, tag="ps",
                            name=f"ps_{rep}_{nt}_{mc}")
        for kp in range(KP):
            mm3(ps, wh_t, wl_t, kp, nt % 2, mc, start=(kp == 0),
                stop=(kp == KP - 1))
            if kp == (8 if i == 0 else 0):
                # B-apply mid-group (xat is ready; for the first tile wait
                # a few k-pairs for the vector-engine xat copy): keeps it
                # off the accumulation tail so the drain starts right after
                # the last hi/lo matmul
                b_apply(ps, nt, mc, stop=False)
        if i == 0:
            for snt, smc in sweep:
                b_apply(ps_sw[(snt, smc)], snt, smc, stop=True)
                drain(ps_sw[(snt, smc)], snt, smc, f"s{snt}_{smc}")
        drain(ps, nt, mc, f"m{nt}_{mc}")


def _build_nc(n_reps=1):
    nc = bacc.Bacc("TRN2", target_bir_lowering=False, debug=False,
                   num_devices=N_CORES)
    xh_d = nc.dram_tensor("xh", [P, KP, 2, M_C], F8, kind="ExternalInput").ap()
    xl_d = nc.dram_tensor("xl", [P, KP, 2, M_C], F8, kind="ExternalInput").ap()
    wh_d = nc.dram_tensor("wh", [P, NTP, KP, 2, 256], F8,
                          kind="ExternalInput").ap()
    wl_d = nc.dram_tensor("wl", [P, NTP, KP, 2, 256], F8,
                          kind="ExternalInput").ap()
    ah_d = nc.dram_tensor("lah", [P, KP, 2, RANK], F8, kind="ExternalInput").ap()
    al_d = nc.dram_tensor("lal", [P, KP, 2, RANK], F8, kind="ExternalInput").ap()
    bb_d = nc.dram_tensor("lb", [RANK, D_OUT], BF16, kind="ExternalInput").ap()
    bias_d = nc.dram_tensor("bias", [P, NT], F32, kind="ExternalInput").ap()
    outt_d = nc.dram_tensor("outt", [D_OUT, M_C], F16,
                            kind="ExternalOutput").ap()

    with tile.TileContext(nc) as tc:
        with (
            tc.tile_pool(name="singles", bufs=1) as singles,
            tc.tile_pool(name="wts", bufs=6) as w_pool,
            tc.tile_pool(name="outs", bufs=6) as out_pool,
            tc.tile_pool(name="psum", bufs=8, space="PSUM") as psum_pool,
        ):
            sb = {
                "xh": singles.tile([P, KP, 2, M_C], F8, name="xh"),
                "xl": singles.tile([P, KP, 2, M_C], F8, name="xl"),
                "ah_sb": singles.tile([P, KP, 2, RANK], F8, name="ah_sb"),
                "al_sb": singles.tile([P, KP, 2, RANK], F8, name="al_sb"),
                "bb_sb": singles.tile([RANK, D_OUT], BF16, name="bb_sb"),
                "xat": singles.tile([RANK, M_C], BF16, name="xat"),
                "bias_sb": singles.tile([P, NT], F32, name="bias_sb"),
            }
            # warmup: the PE clock ramps (0.65/1.2 GHz) over the first ~3us
            # of continuous PE activity; burn the ramp on throwaway matmuls
            # over a zeroed scratch tile during the initial DMA wait so real
            # matmuls start at 2.4 GHz. The PSUM bank is recycled by the
            # pool afterwards.
            warm = singles.tile([P, 64], F8, name="warm")
            nc.vector.memset(warm, 0.0)
            wps = psum_pool.tile([P, 512], F32, tag="ps", name="warm_ps")
            for i in range(60):
                nc.tensor.matmul(wps[0:64, 0:64], warm, warm,
                                 start=(i == 0), stop=(i == 59))
            pools = (singles, w_pool, out_pool, psum_pool)
            aps = (xh_d, xl_d, wh_d, wl_d, ah_d, al_d, bb_d, bias_d, outt_d)
            for rep in range(n_reps):
                _emit_body(nc, pools, aps, sb, rep)

    nc.compile()
    return nc


def get_nc():
    global _NC_CACHE
    if _NC_CACHE is None:
        _NC_CACHE = _build_nc()
    return _NC_CACHE


def _split_f8(a, scale=1.0):
    """Return (hi, lo) fp8 e4m3 pair with a*scale ~= hi + lo."""
    s = (a * scale).astype(np.float32)
    hi = s.astype(NPF8)
    lo = (s - hi.astype(np.float32)).astype(NPF8)
    return hi, lo


def make_in_maps(x, W, bias, lora_A, lora_B, scaling):
    x2 = np.asarray(x, dtype=np.float32).reshape(M_FULL, D_IN)
    w = np.asarray(W, dtype=np.float32)
    b = np.ascontiguousarray(np.asarray(bias, dtype=np.float32))
    a = np.asarray(lora_A, dtype=np.float32)
    s = np.float32(np.asarray(scaling).astype(np.float64))

    # W (scaled by SW) split hi/lo, in [p, ntp, kp, ko, n] DoubleRow layout
    wh, wl = _split_f8(w, SW)
    def w_layout(m):
        return np.ascontiguousarray(
            m.reshape(KP, 2, P, NTP, 256).transpose(2, 3, 0, 1, 4))
    wh, wl = w_layout(wh), w_layout(wl)

    # lora_A scaled by SA and split hi/lo, [p, kp, ko, r]
    ahi, alo = _split_f8(a, SA)
    def a_layout(m):
        return np.ascontiguousarray(
            m.reshape(KP, 2, P, RANK).transpose(2, 0, 1, 3))
    ahi, alo = a_layout(ahi), a_layout(alo)
    # s*B in bf16, [r, n] (the SW/SA scales cancel: xat carries SA=64,
    # the drain divides by SW=64)
    bb = (s * np.asarray(lora_B, dtype=np.float32)).astype(ml_dtypes.bfloat16)
    bias_c = np.ascontiguousarray(b.reshape(NT, P).T)

    maps = []
    for c in range(N_CORES):
        xt = np.ascontiguousarray(x2[c * M_C:(c + 1) * M_C].T)  # [d_in, m]
        xhi, xlo = _split_f8(xt)
        def x_layout(m):
            return np.ascontiguousarray(
                m.reshape(KP, 2, P, M_C).transpose(2, 0, 1, 3))
        maps.append({
            "xh": x_layout(xhi),
            "xl": x_layout(xlo),
            "wh": wh,
            "wl": wl,
            "lah": ahi,
            "lal": alo,
            "lb": bb,
            "bias": bias_c,
        })
    return maps


def assemble_output(results):
    """results: list of per-core dicts with 'outt' [D_OUT, M_C]."""
    out = np.concatenate(
        [results[c]["outt"].T.astype(np.float32) for c in range(N_CORES)],
        axis=0)
    return np.ascontiguousarray(out).reshape(BATCH, SEQ, D_OUT)


def kernel(x, W, bias, lora_A, lora_B, scaling):
    nc = get_nc()
    in_maps = make_in_maps(x, W, bias, lora_A, lora_B, scaling)
    res = run_bass_kernel_spmd(nc, in_maps, core_ids=list(range(N_CORES)))
    return assemble_output(res.results)
